# revision 19
# baseline (speedup 1.0000x reference)
"""Trainium2 Bass kernel for the DA-RNN style input-attention LSTM encoder.

Full-input contract: kernel(**inputs) takes the complete (512, 256, 128) X plus
replicated weights, shards batch across 8 NeuronCores (64 rows each), runs one
SPMD Bass program, and gathers the full (512, 256, 128) output.

Per-core dataflow (b = 64, split into 2 pipeline groups of 32):
  preamble: ux^T[s, b, n] = sum_t Ue[t, s] * X[b, t, n]   (PE, Ue stationary)
  per step t:
    hs^T[s, b]  = We^T @ [2h; c]  (+ be + bu)             (PE; h stored doubled,
                                                           We h-rows pre-halved)
    arg[s,b,n]  = ux^T + hs^T (zero-stride broadcast)     (DVE)
    th          = tanh(arg) -> bf16                       (ACT)
    e[(b,n)]    = ve0^T @ th0 + ve1^T @ th1               (PE rank-1, N=512)
    e_sb(b,n)   <- scatter DMA
    p = exp(e), S = accum_out                             (ACT, no max: |e|<=13)
    u = (p * 1/S) * x_t                                   (DVE fused)
    u^T via PE transpose; z^T[g] = W_g^T u^T + U_g^T h2 + b_g  (PE)
    gates via tanh only (sigmoid(x) = .5 + .5 tanh(x/2)); h2 = (tanh_o+1)*tanh(c)
  out[t, m, b] = h2 -> DRAM; host: transpose to (b, t, m) and * 0.5
"""

import numpy as np

B, T, N, M = 512, 256, 128, 128
NCORES = 8
BL = B // NCORES          # 64 batch rows per core
G = 2                     # pipeline groups per core
BG = BL // G              # 32 batch rows per group

_cached = {}


def _build_nc(t_steps=T, debug=False):
    import concourse.bass as bass
    import concourse.tile as tile
    from concourse import bacc, mybir
    from concourse.masks import make_identity

    f32 = mybir.dt.float32
    bf16 = mybir.dt.bfloat16
    AF = mybir.ActivationFunctionType
    OP = mybir.AluOpType

    nc = bacc.Bacc()

    X = nc.declare_dram_parameter("X", [BL, T, N], f32, isOutput=False)
    We = nc.declare_dram_parameter("We", [2 * M, T], f32, isOutput=False)
    be = nc.declare_dram_parameter("be", [T], f32, isOutput=False)
    Ue = nc.declare_dram_parameter("Ue", [T, T], f32, isOutput=False)
    bu = nc.declare_dram_parameter("bu", [T], f32, isOutput=False)
    ve = nc.declare_dram_parameter("ve", [T, 1], f32, isOutput=False)
    W_lstm = nc.declare_dram_parameter("W_lstm", [N, 4 * M], f32, isOutput=False)
    U_lstm = nc.declare_dram_parameter("U_lstm", [M, 4 * M], f32, isOutput=False)
    b_lstm = nc.declare_dram_parameter("b_lstm", [4 * M], f32, isOutput=False)
    out = nc.declare_dram_parameter("out", [t_steps, M, BL], f32, isOutput=True)
    if debug:
        dbg_ux = nc.declare_dram_parameter("dbg_ux", [128, 2, BL, N], f32, isOutput=True)
        dbg_hs = nc.declare_dram_parameter("dbg_hs", [128, 2, BG], f32, isOutput=True)
        dbg_e = nc.declare_dram_parameter("dbg_e", [BG, N], f32, isOutput=True)
        dbg_u = nc.declare_dram_parameter("dbg_u", [BG, N], f32, isOutput=True)
        dbg_z = nc.declare_dram_parameter("dbg_z", [128, 4, BG], f32, isOutput=True)
        dbg_th = nc.declare_dram_parameter("dbg_th", [128, BG, N], bf16, isOutput=True)

    # z^T gate slot order (i, f, o, g) so the three sigmoid gates are contiguous
    GATE_COL = [0, 1, 3, 2]   # slot -> column block of W_lstm/U_lstm/b_lstm

    with tile.TileContext(nc) as tc:
        with tc.tile_pool(name="singles", bufs=1) as singles:
            # ---- resident weights ----
            we_sb = singles.tile([128, 2, T], f32)       # [k_part, k_tile, s]
            nc.sync.dma_start(out=we_sb, in_=We.rearrange("(kt p) s -> p kt s", p=128))
            ue_sb = singles.tile([128, 2, T], f32)
            nc.sync.dma_start(out=ue_sb, in_=Ue.rearrange("(kt p) s -> p kt s", p=128))
            wl_sb = singles.tile([128, 4 * M], f32)
            nc.sync.dma_start(out=wl_sb, in_=W_lstm[:, :])
            ul_sb = singles.tile([128, 4 * M], f32)
            nc.sync.dma_start(out=ul_sb, in_=U_lstm[:, :])
            blstm_sb = singles.tile([1, 4 * M], f32)
            nc.sync.dma_start(out=blstm_sb, in_=b_lstm[None, :])
            be_sb = singles.tile([1, T], f32)
            nc.sync.dma_start(out=be_sb, in_=be[None, :])
            bu_sb = singles.tile([1, T], f32)
            nc.sync.dma_start(out=bu_sb, in_=bu[None, :])
            ve_sb = singles.tile([128, 2], f32)
            nc.sync.dma_start(out=ve_sb, in_=ve.rearrange("(h p) o -> p (h o)", p=128))

            biasrow = singles.tile([1, T], f32)
            nc.vector.tensor_copy(biasrow, be_sb)
            nc.vector.tensor_add(biasrow, biasrow, bu_sb)
            vebf = singles.tile([128, 2], bf16)
            nc.vector.tensor_copy(vebf, ve_sb)
            ones_sb = singles.tile([1, BG], f32)
            nc.vector.memset(ones_sb, 1.0)
            ident = singles.tile([BG, BG], f32)
            make_identity(nc, ident)

            # ux^T resident: [s_part, s_half, b, n]
            ux_sb = singles.tile([128, 2, BL, N], f32)

            # ---- preamble: ux^T = Ue^T @ X^T, per batch row ----
            with (
                tc.tile_pool(name="xin", bufs=6) as xin,
                tc.tile_pool(name="psux", bufs=4, space="PSUM") as psux,
            ):
                for q in range(BL // 4):          # quads of batch rows
                    xbt = []
                    for j in range(4):
                        xb = xin.tile([128, 2, N], f32, tag="xb")
                        nc.sync.dma_start(
                            out=xb,
                            in_=X[q * 4 + j].rearrange("(kt p) n -> p kt n", p=128),
                        )
                        xbt.append(xb)
                    for h in range(2):
                        pq = psux.tile([128, 4 * N], f32)
                        for j in range(4):
                            for kt in range(2):
                                nc.tensor.matmul(
                                    pq[:, j * N:(j + 1) * N],
                                    ue_sb[:, kt, h * 128:(h + 1) * 128],
                                    xbt[j][:, kt, :],
                                    start=(kt == 0),
                                    stop=(kt == 1),
                                )
                        # alternate copy engine to split preamble load
                        cp = nc.vector if (q + h) % 2 == 0 else nc.scalar
                        if cp is nc.vector:
                            cp.tensor_copy(
                                ux_sb[:, h, q * 4:(q + 1) * 4, :].rearrange("p b n -> p (b n)"),
                                pq,
                            )
                        else:
                            cp.copy(
                                ux_sb[:, h, q * 4:(q + 1) * 4, :].rearrange("p b n -> p (b n)"),
                                pq,
                            )

            # ---- recurrent state ----
            with (
                tc.tile_pool(name="state", bufs=3) as state,
                tc.tile_pool(name="hs_ps", bufs=2, space="PSUM") as hs_ps_pool,
                tc.tile_pool(name="e_ps", bufs=3, space="PSUM") as e_ps_pool,
                tc.tile_pool(name="ut_ps", bufs=1, space="PSUM") as ut_ps_pool,
                tc.tile_pool(name="z_ps", bufs=2, space="PSUM") as z_ps_pool,
                tc.tile_pool(name="args", bufs=3) as args_pool,
                tc.tile_pool(name="th", bufs=3) as th_pool,
                tc.tile_pool(name="small", bufs=3) as small,
                tc.tile_pool(name="xt", bufs=6) as xt_pool,
            ):
                prev_h, prev_c = [], []
                for g in range(G):
                    h0 = state.tile([128, BG], f32, tag=f"h{g}")
                    c0 = state.tile([128, BG], f32, tag=f"c{g}")
                    nc.vector.memset(h0, 0.0)
                    nc.vector.memset(c0, 0.0)
                    prev_h.append(h0)
                    prev_c.append(c0)

                for t in range(t_steps):
                    for g in range(G):
                        bsl = slice(g * BG, (g + 1) * BG)
                        # x_t slice for this group (prefetchable)
                        xt_sb = xt_pool.tile([BG, N], f32, tag="xt")
                        nc.sync.dma_start(out=xt_sb, in_=X[bsl, t, :])

                        # hs^T = We^T @ [2h; c] + (be + bu)
                        hs_ps = hs_ps_pool.tile([128, 2, BG], f32)
                        for h in range(2):
                            ssl = slice(h * 128, (h + 1) * 128)
                            nc.tensor.matmul(hs_ps[:, h, :], we_sb[:, 0, ssl],
                                             prev_h[g], start=True, stop=False)
                            nc.tensor.matmul(hs_ps[:, h, :], we_sb[:, 1, ssl],
                                             prev_c[g], start=False, stop=False)
                            nc.tensor.matmul(hs_ps[:, h, :], biasrow[:, ssl],
                                             ones_sb, start=False, stop=True)
                        hs_sb = small.tile([128, 2, BG], f32, tag="hs")
                        nc.vector.tensor_copy(hs_sb, hs_ps)
                        if debug and t == 0 and g == 0:
                            nc.sync.dma_start(out=dbg_ux[:], in_=ux_sb[:])
                            nc.sync.dma_start(out=dbg_hs[:], in_=hs_sb)

                        # broadcast add + tanh, one (s_half) slab at a time
                        ths = []
                        for h in range(2):
                            arg = args_pool.tile([128, BG, N], f32, tag="arg")
                            if h == 0:
                                nc.vector.tensor_tensor(
                                    arg, ux_sb[:, h, bsl, :],
                                    hs_sb[:, h, :, None].broadcast_to((128, BG, N)),
                                    OP.add)
                            elif g == 1:
                                nc.gpsimd.tensor_tensor(
                                    arg, ux_sb[:, h, bsl, :],
                                    hs_sb[:, h, :, None].broadcast_to((128, BG, N)),
                                    OP.add)
                            else:
                                HB = BG // 2
                                b0 = g * BG
                                nc.vector.tensor_tensor(
                                    arg[:, :HB, :],
                                    ux_sb[:, h, b0:b0 + HB, :],
                                    hs_sb[:, h, :HB, None].broadcast_to((128, HB, N)),
                                    OP.add)
                                nc.gpsimd.tensor_tensor(
                                    arg[:, HB:, :],
                                    ux_sb[:, h, b0 + HB:b0 + BG, :],
                                    hs_sb[:, h, HB:, None].broadcast_to((128, HB, N)),
                                    OP.add)
                            th = th_pool.tile([128, BG, N], bf16, tag="th")
                            nc.scalar.activation(th, arg, AF.Tanh)
                            if debug and t == 0 and g == 0 and h == 0:
                                nc.sync.dma_start(out=dbg_th[:], in_=th)
                            ths.append(th)

                        # e = ve^T @ th (rank-1 stationary, N=512 chunks).
                        # 4 chunk-rows land on partitions {0,32,64,96} of one
                        # PSUM bank via column-group placement, then one
                        # full-partition copy + small scatter DMAs.
                        e_sb = small.tile([BG, N], f32, tag="e")
                        CH = 4  # batch rows per 512-wide chunk
                        for q in range(2):
                            e_ps = e_ps_pool.tile([128, CH * N], f32)
                            for j in range(4):
                                c = q * 4 + j
                                nc.tensor.matmul(
                                    e_ps[32 * j:32 * j + 1, :], vebf[:, 0:1],
                                    ths[0][:, c * CH:(c + 1) * CH, :],
                                    start=True, stop=False,
                                    tile_position=(0, 32 * j))
                                nc.tensor.matmul(
                                    e_ps[32 * j:32 * j + 1, :], vebf[:, 1:2],
                                    ths[1][:, c * CH:(c + 1) * CH, :],
                                    start=False, stop=True,
                                    tile_position=(0, 32 * j))
                            e_flat = small.tile([128, CH * N], f32, tag="eflat")
                            nc.vector.tensor_copy(e_flat, e_ps)
                            nc.sync.dma_start(
                                out=e_sb[q * 16:(q + 1) * 16, :],
                                in_=e_flat[::32, :],
                            )

                        # softmax (logits bounded; skip max-subtract), fold 1/S in
                        if debug and t == 0 and g == 0:
                            nc.sync.dma_start(out=dbg_e[:], in_=e_sb)
                        p_sb = small.tile([BG, N], f32, tag="p")
                        S_sb = small.tile([BG, 1], f32, tag="S")
                        nc.scalar.activation(p_sb, e_sb, AF.Exp, accum_out=S_sb)
                        r_sb = small.tile([BG, 1], f32, tag="r")
                        nc.vector.reciprocal(r_sb, S_sb)
                        u_sb = small.tile([BG, N], f32, tag="u")
                        nc.vector.scalar_tensor_tensor(
                            u_sb, p_sb, r_sb, xt_sb, OP.mult, OP.mult)

                        if debug and t == 0 and g == 0:
                            nc.sync.dma_start(out=dbg_u[:], in_=u_sb)
                        # u^T via PE transpose
                        ut_ps = ut_ps_pool.tile([N, BG], f32)
                        nc.tensor.transpose(ut_ps, u_sb, ident)
                        ut_sb = small.tile([N, BG], f32, tag="ut")
                        nc.vector.tensor_copy(ut_sb, ut_ps)

                        # z^T per gate slot (i, f, o, g)
                        z_ps = z_ps_pool.tile([128, 4, BG], f32)
                        for slot in range(4):
                            gc = GATE_COL[slot]
                            csl = slice(gc * M, (gc + 1) * M)
                            nc.tensor.matmul(z_ps[:, slot, :], wl_sb[:, csl],
                                             ut_sb, start=True, stop=False)
                            nc.tensor.matmul(z_ps[:, slot, :], ul_sb[:, csl],
                                             prev_h[g], start=False, stop=False)
                            nc.tensor.matmul(z_ps[:, slot, :], blstm_sb[:, csl],
                                             ones_sb, start=False, stop=True)

                        if debug and t == 0 and g == 0:
                            zdump = small.tile([128, 4, BG], f32, tag="zdump")
                            nc.vector.tensor_copy(zdump, z_ps)
                            nc.sync.dma_start(out=dbg_z[:], in_=zdump)
                        # gates: sigmoid(x) = .5 + .5*tanh(x/2); the x/2 for
                        # i,f,o is pre-folded into W/U/b on the host, so one
                        # Tanh covers all four slots.
                        t_all = small.tile([128, 4, BG], f32, tag="tifo")
                        nc.scalar.activation(t_all, z_ps, AF.Tanh)
                        t_ifo = t_all[:, 0:3, :]
                        t_g = t_all[:, 3, :]

                        a_sb = small.tile([128, BG], f32, tag="ga")
                        nc.vector.scalar_tensor_tensor(
                            a_sb, t_ifo[:, 1, :], 1.0, prev_c[g], OP.add, OP.mult)
                        b_sb = small.tile([128, BG], f32, tag="gb")
                        nc.vector.scalar_tensor_tensor(
                            b_sb, t_ifo[:, 0, :], 1.0, t_g, OP.add, OP.mult)
                        c_new = state.tile([128, BG], f32, tag=f"c{g}")
                        nc.vector.scalar_tensor_tensor(
                            c_new, a_sb, 0.5, b_sb, OP.mult, OP.add)
                        tc_sb = small.tile([128, BG], f32, tag="tc")
                        nc.scalar.activation(tc_sb, c_new, AF.Tanh, scale=0.5)
                        h_new = state.tile([128, BG], f32, tag=f"h{g}")
                        nc.vector.scalar_tensor_tensor(
                            h_new, t_ifo[:, 2, :], 1.0, tc_sb, OP.add, OP.mult)

                        nc.sync.dma_start(out=out[t, :, bsl], in_=h_new)

                        prev_h[g] = h_new
                        prev_c[g] = c_new
    nc.finalize()
    return nc


LAST_RESULTS = None


def kernel(_trace=False, **inputs):
    global LAST_RESULTS
    import os
    if not _trace:
        # the axon NTFF trace hook module is absent in this container; make
        # sure an inherited BASS_TRACE env can't route us into that path
        os.environ["BASS_NEVER_TRACE"] = "1"
    from concourse.bass_utils import run_bass_kernel_spmd

    if "nc" not in _cached:
        _cached["nc"] = _build_nc()
    nc = _cached["nc"]

    # host-side weight prep (h stored doubled; i/f/o sigmoid input scale
    # folded into the LSTM weights)
    We_eff = 0.5 * np.array(inputs["We"], np.float32)
    W_eff = np.array(inputs["W_lstm"], np.float32, copy=True)
    U_eff = 0.5 * np.asarray(inputs["U_lstm"], np.float32)
    b_eff = np.array(inputs["b_lstm"], np.float32, copy=True)
    for gc in (0, 1, 3):          # i, f, o column blocks
        W_eff[:, gc * M:(gc + 1) * M] *= 0.5
        U_eff[:, gc * M:(gc + 1) * M] *= 0.5
        b_eff[gc * M:(gc + 1) * M] *= 0.5
    prepped = {"We": We_eff, "W_lstm": W_eff, "U_lstm": U_eff, "b_lstm": b_eff,
               "be": np.asarray(inputs["be"], np.float32),
               "bu": np.asarray(inputs["bu"], np.float32),
               "Ue": np.asarray(inputs["Ue"], np.float32),
               "ve": np.asarray(inputs["ve"], np.float32)}
    in_maps = []
    for c in range(NCORES):
        m = {"X": np.ascontiguousarray(inputs["X"][c * BL:(c + 1) * BL])}
        for k, v in prepped.items():
            m[k] = np.ascontiguousarray(v)
        in_maps.append(m)

    res = run_bass_kernel_spmd(nc, in_maps, core_ids=list(range(NCORES)),
                               trace=_trace)
    LAST_RESULTS = res
    full = np.empty((B, T, M), np.float32)
    for c in range(NCORES):
        o = res.results[c]["out"]          # (T, M, BL), h doubled
        full[c * BL:(c + 1) * BL] = 0.5 * o.transpose(2, 0, 1)
    return full
